# revision 15
# baseline (speedup 1.0000x reference)
"""CapsuleLayer forward (squash + per-capsule matmul) on 8 Trainium2 cores.

Reference computation (all fp32):
    x  = inputs.reshape(B, 1152, 8)
    pc = squash(x)                              # per-(b,n) over k=8
    u_hat[b,n,j,d] = sum_k W[0,n,j,d,k] * pc[b,n,k]
    out = u_hat[..., None]                      # [B, 1152, 10, 16, 1]

Sharding: capsule dim (n=1152) split 144-per-core across 8 cores; every core
keeps the full batch (B=512).  Zero cross-device communication.

Per-core kernel:
  - squash on DVE/ACT in natural [b, (n,k)] layout, 128-batch chunks
  - pc transposed to [ck, b] via PE transpose, 16-cap groups (128 rows)
  - W host-packed into 4-cap block-diagonal [32, 640] tiles (engines need
    32-partition-aligned starts), assembled once on-device into 9 resident
    block-diagonal [128, 16*160] SBUF tiles
  - matmul out[b, (c,jd)] = pcT.T @ Wblk  (K=128, M=128, N=512), fp32r
  - PSUM->SBUF on DVE, 1.31 MB HWDGE DMA stores
"""

import os
from contextlib import ExitStack

import numpy as np

import concourse.bacc as bacc
import concourse.bass as bass  # noqa: F401  (AP helpers)
import concourse.mybir as mybir
import concourse.tile as tile
from concourse.bass_utils import run_bass_kernel_spmd
from concourse.masks import make_identity

N_CORES = 8
B = 512
N_CAPS = 1152
K = 8
JD = 160  # 10*16
CAPS_PER_CORE = N_CAPS // N_CORES  # 144
GROUP_CAPS = 16  # caps per matmul group -> K=128
N_GROUPS = CAPS_PER_CORE // GROUP_CAPS  # 9
GROUP_COLS = GROUP_CAPS * JD  # 2560
N_CHUNK = 512  # matmul moving-dim tile (one PSUM bank of fp32)
N_SPLITS = GROUP_COLS // N_CHUNK  # 5
P = 128
B_CHUNKS = B // P  # 4
EPS = 1e-07
SUB_CAPS = 4  # caps per 32-partition diag sub-block
SUB_P = SUB_CAPS * K  # 32
SUB_COLS = SUB_CAPS * JD  # 640
N_SUBS = GROUP_CAPS // SUB_CAPS  # 4

F32 = mybir.dt.float32
# fp32r streams the PE at 1 cycle/row (vs 4 for plain fp32) for N>=256.
MM_DTYPE = mybir.dt.float32r if os.environ.get("CAPS_MM", "f32r") == "f32r" else F32


def build_program():
    nc = bacc.Bacc("TRN2", debug=False, num_devices=N_CORES)
    x = nc.dram_tensor("x", [B, CAPS_PER_CORE * K], F32, kind="ExternalInput").ap()
    wt = nc.dram_tensor(
        "wt", [CAPS_PER_CORE * K, SUB_COLS], F32, kind="ExternalInput"
    ).ap()
    out = nc.dram_tensor(
        "out", [B, CAPS_PER_CORE * JD], F32, kind="ExternalOutput"
    ).ap()

    with tile.TileContext(nc) as tc, ExitStack() as ctx:
        consts = ctx.enter_context(tc.tile_pool(name="consts", bufs=1))
        wload = ctx.enter_context(tc.tile_pool(name="wload", bufs=2))
        wblk_pool = ctx.enter_context(tc.tile_pool(name="wblk", bufs=1))
        xpool = ctx.enter_context(tc.tile_pool(name="xpool", bufs=2))
        pcpool = ctx.enter_context(tc.tile_pool(name="pcpool", bufs=2))
        stats = ctx.enter_context(tc.tile_pool(name="stats", bufs=2))
        pct_pool = ctx.enter_context(tc.tile_pool(name="pct", bufs=3))
        ost_pool = ctx.enter_context(tc.tile_pool(name="ost", bufs=3))
        psum_t = ctx.enter_context(tc.tile_pool(name="psum_t", bufs=2, space="PSUM"))
        psum_m = ctx.enter_context(tc.tile_pool(name="psum_m", bufs=4, space="PSUM"))

        identity = consts.tile([P, P], F32)
        make_identity(nc, identity)
        eps_tile = consts.tile([P, 1], F32)
        nc.vector.memset(eps_tile, EPS)
        zero_col = consts.tile([P, 1], F32)
        nc.vector.memset(zero_col, 0.0)

        # Assemble resident block-diagonal weight tiles from host-packed
        # [32, 640] diagonal sub-blocks, once.
        wblk = []
        for g in range(N_GROUPS):
            wt_tile = wload.tile([P, SUB_COLS], F32)
            nc.scalar.dma_start(out=wt_tile, in_=wt[g * P : (g + 1) * P, :])
            wb = wblk_pool.tile([P, GROUP_COLS], MM_DTYPE, tag=f"wblk{g}")
            # f32r Memset fails walrus ISA codegen; a rounding DVE copy from
            # a zero column is the legal way to clear an fp32r tile.
            nc.vector.tensor_copy(out=wb, in_=zero_col.broadcast_to([P, GROUP_COLS]))
            for q in range(N_SUBS):
                nc.vector.tensor_copy(
                    out=wb[
                        q * SUB_P : (q + 1) * SUB_P,
                        q * SUB_COLS : (q + 1) * SUB_COLS,
                    ],
                    in_=wt_tile[q * SUB_P : (q + 1) * SUB_P, :],
                )
            wblk.append(wb)

        for bi in range(B_CHUNKS):
            xt = xpool.tile([P, CAPS_PER_CORE, K], F32)
            nc.scalar.dma_start(
                out=xt,
                in_=x[bi * P : (bi + 1) * P, :].rearrange("b (c k) -> b c k", k=K),
            )
            # squash: scale[b,c] = sq/(1+sq) / sqrt(sq+eps), pc = x*scale
            x2 = xpool.tile([P, CAPS_PER_CORE, K], F32)
            nc.vector.tensor_mul(x2, xt, xt)
            sq = stats.tile([P, CAPS_PER_CORE], F32)
            nc.vector.reduce_sum(out=sq, in_=x2, axis=mybir.AxisListType.X)
            sn = stats.tile([P, CAPS_PER_CORE], F32)
            nc.scalar.activation(
                out=sn, in_=sq, func=mybir.ActivationFunctionType.Sqrt,
                bias=eps_tile, scale=1.0,
            )
            rn = stats.tile([P, CAPS_PER_CORE], F32)
            nc.vector.reciprocal(rn, sn)
            t1 = stats.tile([P, CAPS_PER_CORE], F32)
            nc.scalar.add(t1, sq, 1.0)
            r2 = stats.tile([P, CAPS_PER_CORE], F32)
            nc.vector.reciprocal(r2, t1)
            m1 = stats.tile([P, CAPS_PER_CORE], F32)
            nc.vector.tensor_mul(m1, sq, rn)
            scale = stats.tile([P, CAPS_PER_CORE], F32)
            nc.vector.tensor_mul(scale, m1, r2)
            pc = pcpool.tile([P, CAPS_PER_CORE, K], F32)
            nc.vector.tensor_mul(
                pc, xt, scale.unsqueeze(2).broadcast_to([P, CAPS_PER_CORE, K])
            )
            pc_flat = pc.rearrange("p c k -> p (c k)")

            for g in range(N_GROUPS):
                pst = psum_t.tile([P, P], F32)
                nc.tensor.transpose(
                    pst, pc_flat[:, g * P : (g + 1) * P], identity
                )
                pcT = pct_pool.tile([P, P], MM_DTYPE)
                nc.vector.tensor_copy(pcT, pst)
                ost = ost_pool.tile([P, GROUP_COLS], F32)
                for s in range(N_SPLITS):
                    pm = psum_m.tile([P, N_CHUNK], F32)
                    nc.tensor.matmul(
                        pm,
                        lhsT=pcT,
                        rhs=wblk[g][:, s * N_CHUNK : (s + 1) * N_CHUNK],
                        start=True,
                        stop=True,
                    )
                    nc.vector.tensor_copy(
                        ost[:, s * N_CHUNK : (s + 1) * N_CHUNK], pm
                    )
                nc.sync.dma_start(
                    out=out[
                        bi * P : (bi + 1) * P,
                        g * GROUP_COLS : (g + 1) * GROUP_COLS,
                    ],
                    in_=ost,
                )
    nc.compile()
    return nc


_PROGRAM = None


def _get_program():
    global _PROGRAM
    if _PROGRAM is None:
        _PROGRAM = build_program()
    return _PROGRAM


def shard_inputs(inputs: np.ndarray, W: np.ndarray) -> list[dict[str, np.ndarray]]:
    # W -> k-major [n, k, jd], then packed as 4-cap diagonal sub-blocks:
    # wtb[(g,q,ci,k), ci*JD+jd] = W[0][n, jd, k]; zeros off-diagonal.  A
    # 16-cap group's 4 sub-blocks stack into one dense [128, 640] DMA load.
    wt_kmaj = np.asarray(W[0], dtype=np.float32).reshape(N_CAPS, JD, K)
    wt_kmaj = wt_kmaj.transpose(0, 2, 1)  # [n, k, jd]
    n_sub_total = N_CAPS // SUB_CAPS
    sub = wt_kmaj.reshape(n_sub_total, SUB_CAPS, K, JD)
    wtb = np.zeros((n_sub_total, SUB_CAPS, K, SUB_COLS), dtype=np.float32)
    for ci in range(SUB_CAPS):
        wtb[:, ci, :, ci * JD : (ci + 1) * JD] = sub[:, ci]
    wtb = wtb.reshape(N_CAPS * K, SUB_COLS)
    in_maps = []
    for i in range(N_CORES):
        c0 = i * CAPS_PER_CORE
        in_maps.append(
            {
                "x": np.ascontiguousarray(
                    inputs[:, c0 * K : (c0 + CAPS_PER_CORE) * K], dtype=np.float32
                ),
                "wt": np.ascontiguousarray(
                    wtb[c0 * K : (c0 + CAPS_PER_CORE) * K]
                ),
            }
        )
    return in_maps


def unshard_output(results: list[dict[str, np.ndarray]]) -> np.ndarray:
    full = np.empty((B, N_CAPS, JD), dtype=np.float32)
    for i in range(N_CORES):
        c0 = i * CAPS_PER_CORE
        full[:, c0 : c0 + CAPS_PER_CORE, :] = results[i]["out"].reshape(
            B, CAPS_PER_CORE, JD
        )
    return full.reshape(B, N_CAPS, 10, 16, 1)


def kernel(inputs: np.ndarray, W: np.ndarray) -> np.ndarray:
    nc = _get_program()
    in_maps = shard_inputs(np.asarray(inputs), np.asarray(W))
    res = run_bass_kernel_spmd(nc, in_maps, core_ids=list(range(N_CORES)))
    return unshard_output(res.results)
